# revision 1
# baseline (speedup 1.0000x reference)
"""LocalConvolution via the segmented-MAC custom DVE op (fp16, 2X_1PORT).

Sharding: 8 cores = (batch n in 4) x (H-half in 2); per core [64, 64, 128].
Partitions = (weight-channel j in 8) x (4-row block pc in 16).

Per (kernel-row i, pixel-parity): one MAC_SEG6_ANT scan over 264 segments of
6 consecutive fp16 x elements (5 taps + 1 zero-weight pad tap -> even fp16
pairs), emitting one row-sum per segment. Odd pixels use a host-shifted x
copy so every pair read is 4B-aligned. The 5 row-sums per parity are summed
on GPSIMD (last block on DVE to cut the tail); fp16 outputs are reassembled
and upcast on the host.
"""

import os

import numpy as np

try:
    import concourse.bass as bass
except ImportError:
    import sys

    for p in ("/opt/trn_rl_repo", "/root/.axon_site/_ro/trn_rl_repo"):
        if p not in sys.path:
            sys.path.insert(0, p)
    import concourse.bass as bass
import concourse.mybir as mybir
from concourse import tile
from concourse.bass_utils import run_bass_kernel_spmd


def _split_multi_waits(nc):
    n_split = 0
    for fn in nc.m.functions:
        for bb in fn.blocks:
            new_insts = []
            for inst in bb.instructions:
                si = inst.sync_info
                if si is not None and len(si.on_wait) > 1:
                    waits = list(si.on_wait)
                    for k, w in enumerate(waits[:-1]):
                        n_split += 1
                        new_insts.append(
                            mybir.InstNoOp(
                                name=f"{inst.name}_w{k}",
                                engine=inst.engine,
                                sync_info=mybir.SyncInfo(
                                    on_wait=[w], on_update=[]
                                ),
                                bass_nofuse=True,
                            )
                        )
                    inst.sync_info = mybir.SyncInfo(
                        on_wait=[waits[-1]], on_update=list(si.on_update)
                    )
                new_insts.append(inst)
            bb.instructions = new_insts
    return n_split


def _register_mac_seg6(use_2x=True):
    from concourse import dve_ops
    from concourse.dve_spec import AluOp, Spec, Src0, Src1, scan
    from concourse.dve_table_gen import dve_ver_for
    from concourse.dve_uop import (
        ENABLE,
        AluInp,
        DelayInp,
        DveOpSpec,
        InpSel,
        OutPath,
        OutSel,
        Trigger,
        UopConfig,
    )

    name = "MAC_SEG6_ANT"

    PD = AluInp.PREV_DELAY_0

    # FSM mirrors the stock subdim machine (TENSOR_PAGED_MASK): a one-cycle
    # non-consuming seed (d <- 0), a steady state that accumulates and (via
    # write_subdim_last) emits d only at the last element of each [.., 6]
    # sub-dim row, and a one-element step state entered at each SUB_DIM_DONE
    # that restarts the accumulator with the new segment's first product.
    def _state(kind, two):
        u = UopConfig()
        u.enable_input(InpSel.SRC_0, 1)  # -> delay chain 0
        u.enable_input(InpSel.SRC_1, 2)  # -> delay chain 1
        if two:
            u.enable_input(InpSel.SRC_0_HI, 3)  # -> chain 2
            u.enable_input(InpSel.SRC_1_HI, 4)  # -> chain 3
            u.enable_input(InpSel.ZERO, 5)  # -> chain 4 (seed value)
            zero_lane, m0_lane, acc_st = 4, 5, 3
            chains = (0, 1, 2, 3, 4)
        else:
            u.enable_input(InpSel.ZERO, 3)  # -> chain 2 (seed value)
            zero_lane, m0_lane, acc_st = 2, None, 1
            chains = (0, 1, 2)
        for st in range(8):
            dp = u.datapath_config[st]
            dp.pass_through_delay(*chains)
            if st == 0:
                dp.enable_alu(AluOp.MULTIPLY, AluInp(PD + 0), AluInp(PD + 1))
            elif two and st == 1:
                dp.enable_alu(AluOp.MULTIPLY, AluInp(PD + 2), AluInp(PD + 3))
                dp.enable_delay_from_src(DelayInp.PREV_ALU_OUT, m0_lane)  # m0
            elif two and st == 2:
                dp.enable_alu(
                    AluOp.ADD, AluInp.PREV_ALU_OUT, AluInp(PD + m0_lane)
                )
            elif st == acc_st:
                if kind == "seed":  # d <- 0
                    dp.enable_alu(
                        AluOp.BYPASS, AluInp(PD + zero_lane), AluInp(PD + zero_lane)
                    )
                elif kind == "step":  # segment start: d <- m
                    dp.enable_alu(
                        AluOp.BYPASS, AluInp.PREV_ALU_OUT, AluInp.PREV_ALU_OUT
                    )
                else:  # steady: d <- d + m (same-stage feedback)
                    dp.enable_alu(
                        AluOp.ADD, AluInp.CURR_ALU_OUT, AluInp.PREV_ALU_OUT
                    )
            else:
                dp.enable_alu(
                    AluOp.BYPASS, AluInp.PREV_ALU_OUT, AluInp.PREV_ALU_OUT
                )
        if kind == "seed":
            u.trigger = (Trigger.COUNT, Trigger.NONE, Trigger.NONE)
            u.next_uop = (1, 0, 0)
            u.repeat_count = 1
        else:
            u.enable_output(OutSel.ALU_OUT, OutPath.WR0_LO)
            u.out_last_subdim_enable = ENABLE
            u.require_inp0 = ENABLE
            u.require_inp1 = ENABLE
            if kind == "steady":
                u.trigger = (
                    Trigger.SRC_TENSOR_DONE,
                    Trigger.SUB_DIM_DONE,
                    Trigger.NONE,
                )
                u.next_uop = (0, 2, 0)
            else:  # step
                u.trigger = (
                    Trigger.SRC_TENSOR_DONE,
                    Trigger.SUB_DIM_DONE,
                    Trigger.COUNT,
                )
                u.next_uop = (0, 2, 1)
                u.repeat_count = 1
        return u

    base_uops = [_state(k, False) for k in ("seed", "steady", "step")]
    # pad to the 2x program's state count (table-gen requires equal lengths);
    # the extra states are unreachable
    base_uops += [UopConfig(), UopConfig()]

    # --- 2X_1PORT program -------------------------------------------------
    # Stock convention (decoded from the gen3 table): 2x programs enable BOTH
    # write0 ports and emit two dst elements per write-cycle. Since one
    # segment sum completes every 3 pair-cycles, segments are processed in
    # A/B pairs: segment A's sum is parked in the stage-4 swap flop (captured
    # every steadyA cycle; the last capture is the boundary value) and both
    # sums are written LO/HI at segment B's subdim-last cycle.
    # Datapath (pairs): st0 m0=lo0*lo1; st1 m1=hi0*hi1 (m0 -> lane4);
    # st2 s=m0+m1; st3 d (seed 0 / reset s / accumulate);
    # st4 A-phase: park d in swap; B-phase: read swap, capture d -> lane5.
    def _state2(kind):
        u = UopConfig()
        u.enable_input(InpSel.SRC_0, 1)  # -> chain 0
        u.enable_input(InpSel.SRC_1, 2)  # -> chain 1
        u.enable_input(InpSel.SRC_0_HI, 3)  # -> chain 2
        u.enable_input(InpSel.SRC_1_HI, 4)  # -> chain 3
        for st in range(8):
            dp = u.datapath_config[st]
            dp.pass_through_delay(0, 1, 2, 3)
            if st >= 2:
                dp.pass_through_delay(4)
            if st >= 5 and kind == "steadyB":
                dp.pass_through_delay(5)
            if st == 0:
                dp.enable_alu(AluOp.MULTIPLY, AluInp(PD + 0), AluInp(PD + 1))
            elif st == 1:
                dp.enable_alu(AluOp.MULTIPLY, AluInp(PD + 2), AluInp(PD + 3))
                dp.enable_delay_from_src(DelayInp.PREV_ALU_OUT, 4)  # m0
            elif st == 2:
                dp.enable_alu(AluOp.ADD, AluInp.PREV_ALU_OUT, AluInp(PD + 4))
            elif st == 3:
                if kind == "seed":  # d <- s - s = 0
                    dp.enable_alu(
                        AluOp.SUBTRACT, AluInp.PREV_ALU_OUT, AluInp.PREV_ALU_OUT
                    )
                elif kind in ("stepA", "stepB"):  # d <- s
                    dp.enable_alu(
                        AluOp.BYPASS, AluInp.PREV_ALU_OUT, AluInp.PREV_ALU_OUT
                    )
                else:  # d <- d + s
                    dp.enable_alu(
                        AluOp.ADD, AluInp.CURR_ALU_OUT, AluInp.PREV_ALU_OUT
                    )
            elif st == 4:
                if kind == "steadyB":
                    # read parked A-sum; park B's d on lane 5 for the HI write
                    dp.enable_alu(
                        AluOp.BYPASS, AluInp.CURR_SWAP_OUT, AluInp.CURR_SWAP_OUT
                    )
                    dp.enable_delay_from_src(DelayInp.PREV_ALU_OUT, 5)
                else:
                    dp.enable_alu(
                        AluOp.BYPASS, AluInp.PREV_ALU_OUT, AluInp.PREV_ALU_OUT
                    )
                    if kind == "steadyA":
                        dp.swap_enable = ENABLE  # park d (last capture = A-sum)
            else:
                dp.enable_alu(
                    AluOp.BYPASS, AluInp.PREV_ALU_OUT, AluInp.PREV_ALU_OUT
                )
        if kind != "seed":
            u.require_inp0 = ENABLE
            u.require_inp1 = ENABLE
        if kind == "steadyB":
            u.enable_output(OutSel.ALU_OUT, OutPath.WR0_LO)  # A-sum
            u.enable_output(OutSel.DELAY_5, OutPath.WR0_HI)  # B-sum
            u.out_last_subdim_enable = ENABLE
        if kind == "seed":
            u.trigger = (Trigger.COUNT, Trigger.NONE, Trigger.NONE)
            u.next_uop = (1, 0, 0)
            u.repeat_count = 1
        elif kind == "steadyA":
            u.trigger = (
                Trigger.SRC_TENSOR_DONE, Trigger.SUB_DIM_DONE, Trigger.NONE,
            )
            u.next_uop = (0, 2, 0)
        elif kind == "stepB":
            u.trigger = (
                Trigger.SRC_TENSOR_DONE, Trigger.SUB_DIM_DONE, Trigger.COUNT,
            )
            u.next_uop = (0, 4, 3)
            u.repeat_count = 1
        elif kind == "steadyB":
            u.trigger = (
                Trigger.SRC_TENSOR_DONE, Trigger.SUB_DIM_DONE, Trigger.NONE,
            )
            u.next_uop = (0, 4, 0)
        else:  # stepA
            u.trigger = (
                Trigger.SRC_TENSOR_DONE, Trigger.SUB_DIM_DONE, Trigger.COUNT,
            )
            u.next_uop = (0, 2, 1)
            u.repeat_count = 1
        return u

    two_uops = (
        [_state2(k) for k in ("seed", "steadyA", "stepB", "steadyB", "stepA")]
        if use_2x
        else None
    )

    def _ref(in0, in1, s0, s1, imm2):
        p = in0.shape[0]
        a = np.asarray(in0, np.float32).reshape(p, -1, SEG)
        b = np.asarray(in1, np.float32).reshape(p, -1, SEG)
        return (a * b).sum(axis=-1)

    spec = Spec(body=scan(AluOp.ADD, Src0 * Src1), reference=_ref)
    if name in dve_ops._SUB_OPCODE_FOR_NAME:
        row = dve_ops._SUB_OPCODE_FOR_NAME[name]
        op = next(o for o in dve_ops.OPS if o.name == name)
    else:
        row = dve_ops._CUSTOM_DVE_ROW_BASE + len(dve_ops.OPS)
        assert row < 0x20
        op = None
    shas = {}
    for ver in {dve_ver_for("TRN2"), "v3", "v4"}:
        compiled = DveOpSpec(
            name=name,
            opcode=row,
            uops=base_uops,
            uops_2x=two_uops,
            rd1_en=True,
            perf_max=1 if use_2x else 0,
        )
        dve_ops._COMPILE_CACHE[(name, ver)] = compiled
        shas[ver] = compiled.sha(ver)
    if op is None:
        op = dve_ops.DveOp(name, spec, subdim=True, uops_sha=shas)
        dve_ops.OPS.append(op)
        dve_ops.CUSTOM_DVE_SPECS[name] = spec
        dve_ops._SUB_OPCODE_FOR_NAME[name] = row
    else:
        op.uops_sha.clear()
        op.uops_sha.update(shas)
    return op


def _set_perf_max(nc, val):
    for fn in nc.m.functions:
        for bb in fn.blocks:
            for inst in bb.instructions:
                if isinstance(inst, mybir.InstCustomDveAnt):
                    inst.perf_max = val


N, C, H, W = 4, 64, 128, 128
K, PAD, CW = 5, 2, 8
HO, WO = 128, 128
RH = 64
WP = W + 2 * PAD  # 132
NJ, NPC, RB = 8, 16, 4
SEG = 6
QP = RB * WP // 2  # 264 pixels per parity per partition per group
FREE = QP * SEG  # 1584
XL = (RB + K) * WP  # 1188
F16 = mybir.dt.float16

PHASE = int(os.environ.get("LC_PHASE", "2"))


def _build_program(phase=PHASE, repeat=1):
    nc = bass.Bass()
    mac = _register_mac_seg6(use_2x=True)
    xs_d = nc.declare_dram_parameter("xs", [8, 2, 128, XL], F16, isOutput=False)
    ws_d = nc.declare_dram_parameter("ws", [K, 2, 128, FREE], F16, isOutput=False)
    out_d = nc.declare_dram_parameter("out", [8, 2, 128, QP], F16, isOutput=True)

    xs_a = xs_d[:]
    ws_a = ws_d[:]
    out_a = out_d[:]

    with tile.TileContext(nc) as tc:
        with (
            tc.tile_pool(name="wpool", bufs=1) as wpool,
            tc.tile_pool(name="xpool", bufs=1) as xpool,
            tc.tile_pool(name="opool", bufs=4) as opool,
            tc.tile_pool(name="ogpool", bufs=4) as ogpool,
            tc.tile_pool(name="tpool", bufs=3) as tpool,
        ):
            # All input DMAs are pre-emitted in first-use order (blocks run
            # parity-major): w(0,even), x(g0,even), w(1..4,even) back-to-back
            # (they pace the first block's scans), the remaining even x slabs,
            # then all odd-parity weights and slabs, which arrive during the
            # long even-parity run.
            ngroups = C // CW
            w_tiles = {}
            x_tiles = {}

            def _load_w(i, par):
                wt = wpool.tile([128, FREE], F16, tag=f"w{i}_{par}")
                nc.sync.dma_start(
                    wt[:],
                    ws_a.__replace__(
                        ap=[[FREE, 128], [1, FREE]],
                        offset=(i * 2 + par) * 128 * FREE,
                    ),
                )
                w_tiles[(i, par)] = wt

            def _load_x(g, par):
                xt = xpool.tile([128, XL], F16, tag=f"x{g}_{par}")
                nc.sync.dma_start(
                    xt[:],
                    xs_a.__replace__(
                        ap=[[XL, 128], [1, XL]],
                        offset=(g * 2 + par) * 128 * XL,
                    ),
                )
                x_tiles[(g, par)] = xt

            _load_w(0, 0)
            _load_x(0, 0)
            _load_w(1, 0)
            _load_x(1, 0)
            for i in range(2, K):
                _load_w(i, 0)
            for g in range(2, ngroups):
                _load_x(g, 0)
            for i in range(K):
                _load_w(i, 1)
            for g in range(ngroups):
                _load_x(g, 1)

            def _scan(g, par, i):
                o = opool.tile([128, QP], F16, tag=f"o{par}_{i}")
                xa = x_tiles[(g, par)][:]
                in0 = xa.__replace__(
                    ap=[xa.ap[0], [2, QP], [1, SEG]],
                    offset=xa.offset + i * WP,
                )
                nc.vector._custom_dve(
                    mac, out=o[:], in0=in0, in1=w_tiles[(i, par)][:]
                )
                return o

            # first two even-parity blocks: interleave scan emission so
            # block-1 scans fill block-0's weight-arrival stalls
            pre = {0: [], 1: []}
            order = [(0, 0), (0, 1), (1, 0), (0, 2), (1, 1), (0, 3), (1, 2),
                     (0, 4), (1, 3), (1, 4)]
            for g, i in order:
                pre[g].append((i, _scan(g, 0, i)))
            for g in (0, 1):
                pre[g] = [o for _, o in sorted(pre[g])]

            for par in range(2):
                for g in range(repeat * ngroups):
                    g = g % ngroups
                    if par == 0 and g in (0, 1):
                        os_ = pre[g]
                    else:
                        os_ = [_scan(g, par, i) for i in range(K)]
                    og = ogpool.tile([128, QP], F16, tag=f"og{par}")
                    t2 = tpool.tile([128, QP], F16, tag=f"t{par}")
                    # last block's combine runs on the (idle) DVE to cut the
                    # Pool-only tail
                    eng = (
                        nc.vector
                        if (g == ngroups - 1 and par == 1)
                        else nc.gpsimd
                    )
                    eng.tensor_add(og[:, :], os_[0][:, :], os_[1][:, :])
                    eng.tensor_add(t2[:, :], os_[2][:, :], os_[3][:, :])
                    eng.tensor_add(og[:, :], og[:, :], t2[:, :])
                    eng.tensor_add(og[:, :], og[:, :], os_[4][:, :])
                    nc.sync.dma_start(
                        out_a.__replace__(
                            ap=[[QP, 128], [1, QP]],
                            offset=(g * 2 + par) * 128 * QP,
                        ),
                        og[:],
                    )
    _set_perf_max(nc, 1)
    mybir.codegen_inst_isa_subclasses(nc)
    _split_multi_waits(nc)
    return nc


def _shard_inputs(input, weight):
    input = np.asarray(input, dtype=np.float32)
    weight = np.asarray(weight, dtype=np.float32)
    in_maps = []
    for n in range(N):
        xp = np.pad(input[n], ((0, 0), (PAD, PAD + 1), (PAD, PAD)))  # [64,133,132]
        sw = np.lib.stride_tricks.sliding_window_view(xp, (RB + K), axis=1)
        sw = np.transpose(sw, (0, 1, 3, 2))  # [c, row0, 9, 132]
        wv = weight[n].reshape(NJ, K, K, HO, WO)
        for half in range(2):
            r0 = RH * half
            idx = r0 + np.arange(NPC) * RB
            slab = sw[:, idx]  # [64, 16, 9, 132]
            xe = np.ascontiguousarray(slab.reshape(C // CW, NJ, NPC, XL))
            xe = xe.reshape(8, 128, XL)
            xo = np.zeros_like(xe)
            xo[..., :-1] = xe[..., 1:]
            xs = np.stack([xe, xo], axis=1).astype(np.float16)  # [8, 2, 128, XL]

            warr = wv[:, :, :, r0 : r0 + RH, :].reshape(
                NJ, K, K, NPC, RB, WO // 2, 2
            )  # [j, i, jj, pc, rr, m, par]
            wtmp = np.zeros((K, 2, NJ, NPC, RB, WP // 2, SEG), np.float32)
            wtmp[:, :, :, :, :, : WO // 2, :K] = np.transpose(
                warr, (1, 6, 0, 3, 4, 5, 2)
            )
            ws = wtmp.reshape(K, 2, 128, FREE).astype(np.float16)
            in_maps.append({"xs": xs, "ws": ws})
    return in_maps


def kernel(input, weight):
    nc = _build_program(PHASE)
    in_maps = _shard_inputs(input, weight)
    res = run_bass_kernel_spmd(nc, in_maps, list(range(8)))
    out = np.empty((N, C, HO, WO), dtype=np.float32)
    for k in range(8):
        n, half = divmod(k, 2)
        o = np.asarray(res.results[k]["out"], dtype=np.float32)
        o = o.reshape(8, 2, NJ, NPC, RB, WP // 2)[..., : WO // 2]
        o = np.transpose(o, (0, 2, 3, 4, 5, 1))  # [g, j, pc, rr, m, par]
        out[n, :, RH * half : RH * (half + 1), :] = o.reshape(C, RH, WO)
    return out



# revision 4
# speedup vs baseline: 1.1596x; 1.1596x over previous
"""LocalConvolution via a pad-free pixel-pair segmented-MAC DVE op (fp16, 2x).

Sharding: 8 cores = (batch n in 4) x (H-half in 2); per core [64, 64, 128].
Partitions = (weight-channel j in 8) x (4-row block pc in 16).

Per (kernel-row i, pixel-parity): one MAC_PAIR10_ANT scan over 132 pixel
PAIRS per partition. Each pair (A, B) consumes 10 weights (5 per pixel,
zero padding eliminated) and one shared 10-element x window; the two
5-tap sums share x reads via swap-flop latches inside a 5-state (2x) /
11-state (1x) uOp FSM, emitting the A/B sums as one fp16 write pair.
The 5 row-sums per parity are summed on GPSIMD (last block on DVE to cut
the tail); fp16 outputs are reassembled and upcast on the host.

Weight order per pair: [A0 A1 A2 A3 A4  B2 B0 B1 B3 B4] where Aj/Bj is
the j-th column tap of the even/odd pair member. x window per pair k is
x[4k .. 4k+9] (elements 7..9 are consumed for stream lockstep but unused;
B only needs x[4k+2 .. 4k+6]).
"""

import os

import numpy as np

try:
    import concourse.bass as bass
except ImportError:
    import sys

    for p in ("/opt/trn_rl_repo", "/root/.axon_site/_ro/trn_rl_repo"):
        if p not in sys.path:
            sys.path.insert(0, p)
    import concourse.bass as bass
import concourse.mybir as mybir
from concourse import tile
from concourse.bass_utils import run_bass_kernel_spmd


def _split_multi_waits(nc):
    n_split = 0
    for fn in nc.m.functions:
        for bb in fn.blocks:
            new_insts = []
            for inst in bb.instructions:
                si = inst.sync_info
                if si is not None and len(si.on_wait) > 1:
                    waits = list(si.on_wait)
                    for k, w in enumerate(waits[:-1]):
                        n_split += 1
                        new_insts.append(
                            mybir.InstNoOp(
                                name=f"{inst.name}_w{k}",
                                engine=inst.engine,
                                sync_info=mybir.SyncInfo(
                                    on_wait=[w], on_update=[]
                                ),
                                bass_nofuse=True,
                            )
                        )
                    inst.sync_info = mybir.SyncInfo(
                        on_wait=[waits[-1]], on_update=list(si.on_update)
                    )
                new_insts.append(inst)
            bb.instructions = new_insts
    return n_split


def _register_mac_pair10():
    from concourse import dve_ops
    from concourse.dve_spec import AluOp as SAluOp, Spec, Src0, Src1, scan
    from concourse.dve_table_gen import dve_ver_for
    from concourse.dve_uop import (
        ENABLE,
        AluInp,
        AluOp,
        DelayInp,
        DveOpSpec,
        InpSel,
        OutPath,
        OutSel,
        Trigger,
        UopConfig,
    )

    name = "MAC_PAIR10_ANT"
    PD = AluInp.PREV_DELAY_0

    # --- 2X_1PORT program --------------------------------------------------
    # Chains: 0 = x_lo, 1 = w_lo, 2 = x_hi, 3 = w_hi, 4 = product capture,
    # 5 = late capture (B2 / s / A-emit). Accumulators: A in st6's out flop,
    # B in st7's (CURR_ALU_OUT temporal feedback). Swap latches: x2@st3,
    # x3@st4 (u2), x5@st5 (u3), x6@st2 (u4). Per 5-issue period (one pair):
    #   u1 (x0,x1 | wA0,wA1): s01 = x0w0+x1w1; A <- s01 (seed)
    #   u2 (x2,x3 | wA2,wA3): A += s23; latch x2, x3
    #   u3 (x4,x5 | wA4,wB2): A += x4*wA4 (final); B <- x4*wB2 (seed); latch x5
    #   u4 (x6,x7 | wB0,wB1): B += x2*wB0 + x3*wB1; stash A into lane5@st7
    #   u5 (x8,x9 | wB3,wB4): B += x5*wB3 + x6*wB4; emit (A, B) as LO/HI
    def _u2x(kind):
        u = UopConfig()
        u.enable_input(InpSel.SRC_0, 1)  # -> chain 0 (x_lo)
        u.enable_input(InpSel.SRC_1, 2)  # -> chain 1 (w_lo)
        u.enable_input(InpSel.SRC_0_HI, 3)  # -> chain 2 (x_hi)
        u.enable_input(InpSel.SRC_1_HI, 4)  # -> chain 3 (w_hi)
        u.require_inp0 = ENABLE
        u.require_inp1 = ENABLE
        dp = u.datapath_config
        if kind in ("u1", "u2"):
            dp[0].enable_alu(AluOp.MULTIPLY, AluInp(PD + 0), AluInp(PD + 1))
            dp[0].pass_through_delay(2, 3)  # x_hi, w_hi onward to st1's mul
            dp[1].enable_alu(AluOp.MULTIPLY, AluInp(PD + 2), AluInp(PD + 3))
            dp[1].enable_delay_from_src(DelayInp.PREV_ALU_OUT, 4)
            dp[2].enable_alu(AluOp.ADD, AluInp.PREV_ALU_OUT, AluInp(PD + 4))
            if kind == "u2":
                # carry x_lo to st3, x_hi to st4 for the latches
                dp[0].pass_through_delay(0)
                dp[1].pass_through_delay(0, 2)
                dp[2].pass_through_delay(0, 2)
                dp[3].enable_alu(
                    AluOp.BYPASS, AluInp.PREV_ALU_OUT, AluInp(PD + 0)
                )
                dp[3].swap_enable = ENABLE  # swap@st3 <- x2
                dp[3].pass_through_delay(2)
                dp[4].enable_alu(
                    AluOp.BYPASS, AluInp.PREV_ALU_OUT, AluInp(PD + 2)
                )
                dp[4].swap_enable = ENABLE  # swap@st4 <- x3
            else:
                dp[3].pass_through_alu()
                dp[4].pass_through_alu()
            dp[5].pass_through_alu()
            if kind == "u1":
                dp[6].enable_alu(
                    AluOp.BYPASS, AluInp.PREV_ALU_OUT, AluInp.PREV_ALU_OUT
                )  # A <- s01
            else:
                dp[6].enable_alu(
                    AluOp.ADD, AluInp.CURR_ALU_OUT, AluInp.PREV_ALU_OUT
                )  # A += s23
        elif kind == "u3":
            # st0: A4 = x4*wA4; st1: B2 = x4*wB2 (x_lo reused on both muls)
            dp[0].enable_alu(AluOp.MULTIPLY, AluInp(PD + 0), AluInp(PD + 1))
            dp[0].pass_through_delay(0, 2, 3)
            dp[1].enable_alu(AluOp.MULTIPLY, AluInp(PD + 0), AluInp(PD + 3))
            dp[1].enable_delay_from_src(DelayInp.PREV_ALU_OUT, 4)  # A4
            dp[1].pass_through_delay(2)
            for st in (2, 3, 4):
                dp[st].pass_through_alu()  # pass B2 down
                dp[st].pass_through_delay(2, 4)
            dp[5].enable_alu(
                AluOp.BYPASS, AluInp.PREV_ALU_OUT, AluInp(PD + 2)
            )
            dp[5].swap_enable = ENABLE  # swap@st5 <- x5
            dp[5].pass_through_delay(4)
            dp[6].enable_alu(AluOp.ADD, AluInp.CURR_ALU_OUT, AluInp(PD + 4))
            dp[6].enable_delay_from_src(DelayInp.PREV_ALU_OUT, 5)  # B2
            dp[7].enable_alu(AluOp.BYPASS, AluInp(PD + 5), AluInp(PD + 5))
            # st7 out flop <- B2 (B seed)
        elif kind == "u4":
            # w pair (wB0, wB1); x pair (x6, x7): latch x6, x7 unused.
            dp[0].pass_through_delay(0, 1, 3)
            dp[1].pass_through_delay(0, 1, 3)
            dp[2].enable_alu(
                AluOp.BYPASS, AluInp.PREV_ALU_OUT, AluInp(PD + 0)
            )
            dp[2].swap_enable = ENABLE  # swap@st2 <- x6
            dp[2].pass_through_delay(1, 3)
            dp[3].enable_alu(AluOp.MULTIPLY, AluInp.CURR_SWAP_OUT, AluInp(PD + 1))
            dp[3].pass_through_delay(3)  # B0 = x2*wB0
            dp[4].enable_alu(AluOp.MULTIPLY, AluInp.CURR_SWAP_OUT, AluInp(PD + 3))
            dp[4].enable_delay_from_src(DelayInp.PREV_ALU_OUT, 4)  # B0
            # B1 = x3*wB1
            dp[5].enable_alu(AluOp.ADD, AluInp.PREV_ALU_OUT, AluInp(PD + 4))
            dp[6].enable_delay_from_src(DelayInp.PREV_ALU_OUT, 5)  # s; A held
            dp[7].enable_alu(AluOp.ADD, AluInp.CURR_ALU_OUT, AluInp(PD + 5))
            dp[7].enable_delay_from_src(DelayInp.PREV_ALU_OUT, 5)  # A -> lane5@st7
        elif kind == "u5":
            # w pair (wB3, wB4); x pair consumed but unused.
            dp[0].pass_through_delay(1, 3)
            dp[1].pass_through_delay(1, 3)
            dp[2].enable_alu(AluOp.MULTIPLY, AluInp.CURR_SWAP_OUT, AluInp(PD + 3))
            dp[2].pass_through_delay(1)  # B4 = x6*wB4
            dp[3].pass_through_alu()
            dp[3].pass_through_delay(1)
            dp[4].pass_through_alu()
            dp[4].pass_through_delay(1)
            dp[5].enable_alu(AluOp.MULTIPLY, AluInp.CURR_SWAP_OUT, AluInp(PD + 1))
            dp[5].enable_delay_from_src(DelayInp.PREV_ALU_OUT, 4)  # B4
            # B3 = x5*wB3
            dp[6].enable_alu(AluOp.ADD, AluInp.PREV_ALU_OUT, AluInp(PD + 4))
            # s2 = B3+B4 (clobbers A flop; A already stashed in lane5@st7)
            dp[7].enable_alu(AluOp.ADD, AluInp.CURR_ALU_OUT, AluInp.PREV_ALU_OUT)
            u.enable_output(OutSel.DELAY_5, OutPath.WR0_LO)  # A
            u.enable_output(OutSel.ALU_OUT, OutPath.WR0_HI)  # B
        return u

    def _chain2x(u, succ):
        u.trigger = (Trigger.SRC_TENSOR_DONE, Trigger.COUNT, Trigger.NONE)
        u.next_uop = (0, succ, 0)
        u.repeat_count = 1
        return u

    # index 0 is the entry copy of u1 (index 0 is also IDLE as a next_uop
    # target, so the loop body lives at 1..5)
    two_uops = [
        _chain2x(_u2x("u1"), 2),
        _chain2x(_u2x("u1"), 2),
        _chain2x(_u2x("u2"), 3),
        _chain2x(_u2x("u3"), 4),
        _chain2x(_u2x("u4"), 5),
        _chain2x(_u2x("u5"), 1),
    ]

    # --- 1X program (fallback; also what runs if alignment breaks) ---------
    # Chains: 0 = x, 1 = w, 4/5 = captures. Swap latches: x2@st1, x3@st2,
    # x4@st3, x5@st4, x6@st5. A accumulates in st6, B in st7. A is emitted
    # at i4 (via st7 bypass), B at i9.
    def _u1x(kind):
        u = UopConfig()
        u.enable_input(InpSel.SRC_0, 1)  # -> chain 0 (x)
        u.enable_input(InpSel.SRC_1, 2)  # -> chain 1 (w)
        u.require_inp0 = ENABLE
        u.require_inp1 = ENABLE
        dp = u.datapath_config
        if kind in ("i0", "i1", "i2", "i3", "i4"):
            dp[0].enable_alu(AluOp.MULTIPLY, AluInp(PD + 0), AluInp(PD + 1))
            latch_st = {"i2": 1, "i3": 2, "i4": 3}.get(kind)
            if latch_st is not None:
                for st in range(latch_st):
                    dp[st].pass_through_delay(0)
            for st in range(1, 6):
                if st == latch_st:
                    dp[st].enable_alu(
                        AluOp.BYPASS, AluInp.PREV_ALU_OUT, AluInp(PD + 0)
                    )
                    dp[st].swap_enable = ENABLE
                else:
                    dp[st].pass_through_alu()
            if kind == "i0":
                dp[6].enable_alu(
                    AluOp.BYPASS, AluInp.PREV_ALU_OUT, AluInp.PREV_ALU_OUT
                )
            else:
                dp[6].enable_alu(
                    AluOp.ADD, AluInp.CURR_ALU_OUT, AluInp.PREV_ALU_OUT
                )
            if kind == "i4":
                # A final: mirror it into st7's flop and emit
                dp[7].enable_alu(
                    AluOp.BYPASS, AluInp.PREV_ALU_OUT, AluInp.PREV_ALU_OUT
                )
                u.enable_output(OutSel.ALU_OUT, OutPath.WR0_LO)
        elif kind == "i5":
            # B2 = x4*wB2 at st3; latch x5@st4; B <- B2 (seed)
            for st in (0, 1, 2):
                dp[st].pass_through_delay(0, 1)
            dp[3].enable_alu(AluOp.MULTIPLY, AluInp.CURR_SWAP_OUT, AluInp(PD + 1))
            dp[3].pass_through_delay(0)
            dp[4].enable_alu(
                AluOp.BYPASS, AluInp.PREV_ALU_OUT, AluInp(PD + 0)
            )
            dp[4].swap_enable = ENABLE
            dp[5].pass_through_alu()
            dp[6].enable_delay_from_src(DelayInp.PREV_ALU_OUT, 5)  # B2; A held
            dp[7].enable_alu(AluOp.BYPASS, AluInp(PD + 5), AluInp(PD + 5))
        elif kind == "i6":
            # B0 = x2*wB0 at st1; latch x6@st5; B += B0
            dp[0].pass_through_delay(0, 1)
            dp[1].enable_alu(AluOp.MULTIPLY, AluInp.CURR_SWAP_OUT, AluInp(PD + 1))
            dp[1].pass_through_delay(0)
            for st in (2, 3, 4):
                dp[st].pass_through_alu()
                dp[st].pass_through_delay(0)
            dp[5].enable_alu(
                AluOp.BYPASS, AluInp.PREV_ALU_OUT, AluInp(PD + 0)
            )
            dp[5].swap_enable = ENABLE
            dp[6].enable_delay_from_src(DelayInp.PREV_ALU_OUT, 5)
            dp[7].enable_alu(AluOp.ADD, AluInp.CURR_ALU_OUT, AluInp(PD + 5))
        elif kind in ("i7", "i8", "i9"):
            mul_st = {"i7": 2, "i8": 4, "i9": 5}[kind]
            for st in range(mul_st):
                dp[st].pass_through_delay(1)
            dp[mul_st].enable_alu(
                AluOp.MULTIPLY, AluInp.CURR_SWAP_OUT, AluInp(PD + 1)
            )
            for st in range(mul_st + 1, 6):
                dp[st].pass_through_alu()
            dp[6].enable_delay_from_src(DelayInp.PREV_ALU_OUT, 5)
            dp[7].enable_alu(AluOp.ADD, AluInp.CURR_ALU_OUT, AluInp(PD + 5))
            if kind == "i9":
                u.enable_output(OutSel.ALU_OUT, OutPath.WR0_LO)
        return u

    def _chain1x(u, succ):
        u.trigger = (Trigger.SRC_TENSOR_DONE, Trigger.COUNT, Trigger.NONE)
        u.next_uop = (0, succ, 0)
        u.repeat_count = 1
        return u

    kinds1x = ["i0", "i0", "i1", "i2", "i3", "i4", "i5", "i6", "i7", "i8", "i9"]
    base_uops = [
        _chain1x(_u1x(k), 2 if idx == 0 else (idx + 1) if idx < 10 else 1)
        for idx, k in enumerate(kinds1x)
    ]
    # table-gen requires equal variant lengths; pad 2x with unreachable slots
    two_uops = two_uops + [UopConfig() for _ in range(len(base_uops) - len(two_uops))]

    def _ref(in0, in1, s0, s1, imm2):
        p = in0.shape[0]
        x = np.asarray(in0, np.float32).reshape(p, -1, 10)
        w = np.asarray(in1, np.float32).reshape(p, -1, 10)
        a = (x[..., 0:5] * w[..., 0:5]).sum(axis=-1)
        b = (
            x[..., 4] * w[..., 5]
            + x[..., 2] * w[..., 6]
            + x[..., 3] * w[..., 7]
            + x[..., 5] * w[..., 8]
            + x[..., 6] * w[..., 9]
        )
        out = np.stack([a, b], axis=-1).reshape(p, -1)
        return out

    spec = Spec(body=scan(SAluOp.ADD, Src0 * Src1), reference=_ref)
    if name in dve_ops._SUB_OPCODE_FOR_NAME:
        row = dve_ops._SUB_OPCODE_FOR_NAME[name]
        op = next(o for o in dve_ops.OPS if o.name == name)
    else:
        row = dve_ops._CUSTOM_DVE_ROW_BASE + len(dve_ops.OPS)
        assert row < 0x20
        op = None
    shas = {}
    for ver in {dve_ver_for("TRN2"), "v3", "v4"}:
        compiled = DveOpSpec(
            name=name,
            opcode=row,
            uops=base_uops,
            uops_2x=two_uops,
            rd1_en=True,
            perf_max=1,
        )
        dve_ops._COMPILE_CACHE[(name, ver)] = compiled
        shas[ver] = compiled.sha(ver)
    if op is None:
        op = dve_ops.DveOp(name, spec, subdim=True, uops_sha=shas)
        dve_ops.OPS.append(op)
        dve_ops.CUSTOM_DVE_SPECS[name] = spec
        dve_ops._SUB_OPCODE_FOR_NAME[name] = row
    else:
        op.uops_sha.clear()
        op.uops_sha.update(shas)
    return op


def _set_perf_max(nc, val):
    for fn in nc.m.functions:
        for bb in fn.blocks:
            for inst in bb.instructions:
                if isinstance(inst, mybir.InstCustomDveAnt):
                    inst.perf_max = val


N, C, H, W = 4, 64, 128, 128
K, PAD, CW = 5, 2, 8
HO, WO = 128, 128
RH = 64
WP = W + 2 * PAD  # 132
NJ, NPC, RB = 8, 16, 4
QP = RB * WP // 2  # 264 pixels per parity per partition per group
NPAIR = QP // 2  # 132 pixel pairs
WSEG = 10  # weights per pair (pad-free)
WFREE = NPAIR * WSEG  # 1320
XL = (RB + K) * WP  # 1188
F16 = mybir.dt.float16

PHASE = int(os.environ.get("LC_PHASE", "2"))


def _build_program(phase=PHASE, repeat=1):
    nc = bass.Bass()
    mac = _register_mac_pair10()
    xs_d = nc.declare_dram_parameter("xs", [8, 2, 128, XL], F16, isOutput=False)
    ws_d = nc.declare_dram_parameter("ws", [K, 2, 128, WFREE], F16, isOutput=False)
    out_d = nc.declare_dram_parameter("out", [8, 2, 128, QP], F16, isOutput=True)

    xs_a = xs_d[:]
    ws_a = ws_d[:]
    out_a = out_d[:]

    with tile.TileContext(nc) as tc:
        with (
            tc.tile_pool(name="wpool", bufs=1) as wpool,
            tc.tile_pool(name="xpool", bufs=1) as xpool,
            tc.tile_pool(name="opool", bufs=4) as opool,
            tc.tile_pool(name="ogpool", bufs=4) as ogpool,
            tc.tile_pool(name="tpool", bufs=3) as tpool,
        ):
            # All input DMAs are pre-emitted in first-use order (blocks run
            # parity-major): w(0,even), x(g0,even), w(1..4,even) back-to-back
            # (they pace the first block's scans), the remaining even x slabs,
            # then all odd-parity weights and slabs, which arrive during the
            # long even-parity run.
            ngroups = C // CW
            w_tiles = {}
            x_tiles = {}

            def _load_w(i, par):
                wt = wpool.tile([128, WFREE], F16, tag=f"w{i}_{par}")
                nc.sync.dma_start(
                    wt[:],
                    ws_a.__replace__(
                        ap=[[WFREE, 128], [1, WFREE]],
                        offset=(i * 2 + par) * 128 * WFREE,
                    ),
                )
                w_tiles[(i, par)] = wt

            def _load_x(g, par):
                xt = xpool.tile([128, XL], F16, tag=f"x{g}_{par}")
                nc.sync.dma_start(
                    xt[:],
                    xs_a.__replace__(
                        ap=[[XL, 128], [1, XL]],
                        offset=(g * 2 + par) * 128 * XL,
                    ),
                )
                x_tiles[(g, par)] = xt

            _load_w(0, 0)
            _load_x(0, 0)
            _load_w(1, 0)
            _load_x(1, 0)
            for i in range(2, K):
                _load_w(i, 0)
            for g in range(2, ngroups):
                _load_x(g, 0)
            for i in range(K):
                _load_w(i, 1)
            for g in range(ngroups):
                _load_x(g, 1)

            def _scan(g, par, i):
                o = opool.tile([128, QP], F16, tag=f"o{par}_{i}")
                xa = x_tiles[(g, par)][:]
                in0 = xa.__replace__(
                    ap=[xa.ap[0], [4, NPAIR], [1, WSEG]],
                    offset=xa.offset + i * WP,
                )
                nc.vector._custom_dve(
                    mac, out=o[:], in0=in0, in1=w_tiles[(i, par)][:]
                )
                return o

            # first two even-parity blocks: interleave scan emission so
            # block-1 scans fill block-0's weight-arrival stalls
            pre = {0: [], 1: []}
            order = [(0, 0), (0, 1), (1, 0), (0, 2), (1, 1), (0, 3), (1, 2),
                     (0, 4), (1, 3), (1, 4)]
            for g, i in order:
                pre[g].append((i, _scan(g, 0, i)))
            for g in (0, 1):
                pre[g] = [o for _, o in sorted(pre[g])]

            for par in range(2):
                for g in range(repeat * ngroups):
                    g = g % ngroups
                    if par == 0 and g in (0, 1):
                        os_ = pre[g]
                    else:
                        os_ = [_scan(g, par, i) for i in range(K)]
                    og = ogpool.tile([128, QP], F16, tag=f"og{par}")
                    t2 = tpool.tile([128, QP], F16, tag=f"t{par}")
                    # last block's combine runs on the (idle) DVE to cut the
                    # Pool-only tail
                    eng = (
                        nc.vector
                        if (g == ngroups - 1 and par == 1)
                        else nc.gpsimd
                    )
                    eng.tensor_add(og[:, :], os_[0][:, :], os_[1][:, :])
                    eng.tensor_add(t2[:, :], os_[2][:, :], os_[3][:, :])
                    eng.tensor_add(og[:, :], og[:, :], t2[:, :])
                    eng.tensor_add(og[:, :], og[:, :], os_[4][:, :])
                    nc.sync.dma_start(
                        out_a.__replace__(
                            ap=[[QP, 128], [1, QP]],
                            offset=(g * 2 + par) * 128 * QP,
                        ),
                        og[:],
                    )
    _set_perf_max(nc, 1)
    mybir.codegen_inst_isa_subclasses(nc)
    _split_multi_waits(nc)
    return nc


def _shard_inputs(input, weight):
    input = np.asarray(input, dtype=np.float32)
    weight = np.asarray(weight, dtype=np.float32)
    in_maps = []
    for n in range(N):
        xp = np.pad(input[n], ((0, 0), (PAD, PAD + 1), (PAD, PAD)))  # [64,133,132]
        sw = np.lib.stride_tricks.sliding_window_view(xp, (RB + K), axis=1)
        sw = np.transpose(sw, (0, 1, 3, 2))  # [c, row0, 9, 132]
        wv = weight[n].reshape(NJ, K, K, HO, WO)
        for half in range(2):
            r0 = RH * half
            idx = r0 + np.arange(NPC) * RB
            slab = sw[:, idx]  # [64, 16, 9, 132]
            xe = np.ascontiguousarray(slab.reshape(C // CW, NJ, NPC, XL))
            xe = xe.reshape(8, 128, XL)
            xo = np.zeros_like(xe)
            xo[..., :-1] = xe[..., 1:]
            xs = np.stack([xe, xo], axis=1).astype(np.float16)  # [8, 2, 128, XL]

            warr = wv[:, :, :, r0 : r0 + RH, :].reshape(
                NJ, K, K, NPC, RB, WO // 2, 2
            )  # [j, i, jj, pc, rr, m, par]
            # taps per (i, par, j, pc, rr, within-parity pixel m, col-tap jj)
            wt = np.zeros((K, 2, NJ, NPC, RB, WP // 2, K), np.float32)
            wt[:, :, :, :, :, : WO // 2, :] = np.transpose(
                warr, (1, 6, 0, 3, 4, 5, 2)
            )
            # pack per pixel-pair: [A0..A4, B2, B0, B1, B3, B4]
            wpair = wt.reshape(K, 2, NJ, NPC, RB, WP // 4, 2, K)
            wpp = np.empty((K, 2, NJ, NPC, RB, WP // 4, WSEG), np.float32)
            wpp[..., 0:5] = wpair[..., 0, :]
            wpp[..., 5] = wpair[..., 1, 2]
            wpp[..., 6] = wpair[..., 1, 0]
            wpp[..., 7] = wpair[..., 1, 1]
            wpp[..., 8] = wpair[..., 1, 3]
            wpp[..., 9] = wpair[..., 1, 4]
            ws = wpp.reshape(K, 2, 128, WFREE).astype(np.float16)
            in_maps.append({"xs": xs, "ws": ws})
    return in_maps


def kernel(input, weight):
    nc = _build_program(PHASE)
    in_maps = _shard_inputs(input, weight)
    res = run_bass_kernel_spmd(nc, in_maps, list(range(8)))
    out = np.empty((N, C, HO, WO), dtype=np.float32)
    for k in range(8):
        n, half = divmod(k, 2)
        o = np.asarray(res.results[k]["out"], dtype=np.float32)
        o = o.reshape(8, 2, NJ, NPC, RB, WP // 2)[..., : WO // 2]
        o = np.transpose(o, (0, 2, 3, 4, 5, 1))  # [g, j, pc, rr, m, par]
        out[n, :, RH * half : RH * (half + 1), :] = o.reshape(C, RH, WO)
    return out
